# revision 35
# baseline (speedup 1.0000x reference)
"""FWHT (N=16384, orthonormal) over a (32, 64, 16384) f32 batch on 8 TRN2 cores.

Decomposition: H_16384 = H_128 (x) H_128.  Each length-16384 row reshaped to
X[i, j] (128x128) transforms as Y = H X H / 128.  On the PE (out = lhsT.T @ rhs,
lhsT stationary):
  stage 1 (per row):    lhsT = X_r  (K=i), rhs = H       -> ps1[j, a] = (H X_r)^T[j, a]
  stage 2 (per 8 rows): lhsT = H/128 (K=j), rhs = ps1-in-SBUF batched [j, (r a)]
                        -> ps2[b, (r a)] = Y_r^T[b, a]   (512-col matmuls)
The transposed output layout [b, r, a] is untransposed by the host for free.

Default scheme "fp8" (FWHT_SCHEME=fp16 restores the older variant):
  x: fp8 e3m4 (float8e3), host-pretransposed to [i, (r j)] so every DMA line
     is contiguous.  e3m4 matmuls run at full PE rate (1 col/cycle) and the
     e3m4 x e3m4 -> f32 PSUM stage-1 is exact given the quantized input, so
     the input cast costs no engine time.  Contributes ~1.2e-2 rel error.
  mid: bf16.  DVE drains its share as a strided bitcast copy of the f32
     high halves (bf16-RTZ truncation, DVE's fastest PSUM op at 0.993
     ns/col); ACT converts f32->bf16 at its 0.756 copy rate.  ~+2e-3.
  y: fp16 (PSUM f32 -> SBUF copy, no quant scale; FWHT_FP16OUT=0 restores
     the int8×6/127 output), ~2.4e-4.
  Total measured rel err 1.477e-2 (deterministic; gate is 2e-2).
Per-core HBM traffic: 4.19 MB in + 8.39 MB out (vs 33.6 MB in f32).

Engine budget per 256-row pass (measured via probe_rates.py + component
benches):
  PE     65536 cols @ ~0.42        = 27.3 us  (both stages, 1 col/cycle)
  drain  65536 PSUM cols on DVE+ACT ~= 28.3 us  <- the wall.  Only DVE and
         ACT can read PSUM (GPSIMD/Pool cannot; DMA cannot; PE streams from
         SBUF only), so the mid cast + output drain bound the kernel.
         Split: DVE 28 bf16-high-half mid drains (0.993 ns/col), ACT the
         other 4 mids + all 32 fp16 output copies (0.756 ns/col).
  DMA    12.6 MB @ ~920 GB/s measured =~ 14 us  (no_compute bench: DMA is
         4x faster than the 360 GB/s cost-model figure, so spending HBM
         bytes to relieve the PSUM drain is the right trade)

Sharding: pure data-parallel over the 2048 leading rows -> 256 rows/core.
"""

import os as _os

import numpy as np

import concourse.bass as bass
import concourse.bacc as bacc
import concourse.tile as tile
import concourse.mybir as mybir
from concourse.bass_utils import run_bass_kernel_spmd

N_CORES = 8
R = 256          # rows per core (2048 / 8)
BLK = int(_os.environ.get("FWHT_BLK", "32"))  # rows per DMA block (1 MB fp16 in, 512 KB int8 out)
GRP = 8          # rows per PSUM group (8 * 128 f32 = two 2KB PSUM banks)
NBLK = R // BLK
GPB = BLK // GRP  # groups per block
NGRP_ALL = R // GRP
PREF = int(_os.environ.get("FWHT_PREF", str(max(2, 64 // BLK))))  # in-DMA prefetch depth (blocks)
MERGE_IN = _os.environ.get("FWHT_MERGE_IN", "0") == "1"  # 2-block (2MB) in-DMAs
QAMP = 7.0       # int8 quant range: y in [-QAMP, QAMP]  (fp16 scheme)
QSCALE = 127.0 / QAMP
QAMP8 = 6.0      # fp8 scheme: tighter range (max |y| = 5.46, never clips)
QSCALE8 = 127.0 / QAMP8
# host2 scheme: mid values W = H @ X_r are N(0, 128); clip at 6.5 sigma
MID_AMP = 6.5 * 128.0 ** 0.5
QSCALE2 = 127.0 / MID_AMP
LAG2 = 2         # stage1 -> quant lag (host2 scheme)

_cache = {}
LAST_RESULTS = None


def _hadamard128() -> np.ndarray:
    idx = np.arange(128, dtype=np.uint32)
    bits = idx[:, None] & idx[None, :]
    pop = np.zeros_like(bits)
    for s in range(7):
        pop += (bits >> s) & 1
    return np.where(pop % 2 == 0, np.float32(1.0), np.float32(-1.0)).astype(np.float32)


def _h_input() -> np.ndarray:
    H = _hadamard128()
    return np.concatenate([H, H / np.float32(128.0)], axis=1).astype(np.float16)


def _h_inputs(scheme: str = "fp16") -> dict:
    """Host-side constant-input map keyed by dram tensor name."""
    if scheme == "fp8":
        import ml_dtypes

        H = _hadamard128()
        return {
            "h8": H.astype(ml_dtypes.float8_e3m4),      # stage-1 moving operand
            # stage-2 stationary in bf16 (+-2^-7 exact) to match bf16 mids
            "h": (H / np.float32(128.0)).astype(ml_dtypes.bfloat16),
        }
    return {"h": _h_input()}


def _engine_split(fp16_out: bool = True):
    """Assign the 64 per-pass PSUM-drain ops (32 mid casts + 32 quants,
    GRP*128 cols each) across DVE (0.96 G cols/s) and ACT (1.2 G cols/s)
    so both engines carry equal busy time -- the PSUM drain through these
    two engines is the kernel's binding resource.  GPSIMD/Pool cannot
    access PSUM (BIR verifier).  Greedy in pipeline emission order
    (copy G, quant G-LAG).  Returns (copy_eng[32], quant_eng[32]) with
    entries in {'v','s'}."""
    # measured ns/col (probe_rates.py on this terminal):
    #   copy  (PSUM f32 -> SBUF fp16): DVE 1.181  ACT 0.756
    #   quant (PSUM f32 -> SBUF int8): DVE 1.132  ACT 1.038
    if not fp16_out:
        # int8 out: LP-optimal by comparative advantage: ALL mid casts on
        # ACT, quants mostly on DVE with 5.5/32 on ACT -> 30.7us each.
        copy_eng = ["s"] * NGRP_ALL
        act_quants = {2, 9, 16, 22, 28}      # 5 full quants on ACT
        quant_eng = ["s" if g in act_quants else "v" for g in range(NGRP_ALL)]
        quant_eng[13] = "vs"                  # one split op: half DVE half ACT
        return copy_eng, quant_eng
    # fp16 out + bf16 mid: DVE's fastest PSUM op is the strided bf16
    # high-half copy (0.993 ns/col), so DVE takes 27.5/32 mids; ACT (0.756)
    # takes the other 4.5 mids and ALL 32 output copies.
    # DVE 27.5*1.017us = 28.0us, ACT 36.5*0.774us = 28.25us.
    act_mids = {6, 14, 22, 30}
    copy_eng = ["s" if g in act_mids else "v" for g in range(NGRP_ALL)]
    copy_eng[27] = "vs"                       # one split op: half DVE half ACT
    quant_eng = ["s"] * NGRP_ALL
    return copy_eng, quant_eng


def _build(repeat: int = 1, bench: bool = False, no_compute: bool = False,
           no_dma: bool = False, scheme: str = "fp16", unroll: int = 1):
    nc = bacc.Bacc(
        "TRN2",
        target_bir_lowering=False,
        debug=False,
        num_devices=N_CORES,
    )
    f32 = mybir.dt.float32
    xdt = mybir.dt.float16
    bf16 = mybir.dt.bfloat16
    i8 = mybir.dt.int8
    f8 = mybir.dt.float8e3
    host2 = scheme == "host2"
    fp8 = scheme == "fp8"
    indt = f8 if fp8 else xdt  # x / xt dtype
    f16out = fp8 and _os.environ.get("FWHT_FP16OUT", "1") == "1"
    odt = xdt if f16out else i8  # y output dtype: fp16 for the fp8 scheme
    qscale = QSCALE8 if fp8 else QSCALE

    mdt = bf16 if fp8 else xdt  # mid dtype (bf16 enables DVE high-half drains)
    if fp8:
        h8_d = nc.dram_tensor("h8", [128, 128], f8, kind="ExternalInput").ap()
        h_d = nc.dram_tensor("h", [128, 128], mdt, kind="ExternalInput").ap()
    else:
        h_d = nc.dram_tensor("h", [128, 256], xdt, kind="ExternalInput").ap()
    if bench:
        # Timing-only: x/y live in internal DRAM scratch (same addresses,
        # sizes and DMA patterns), so the PJRT call ships ~64KB instead of
        # ~12MB per core - cuts per-call wall noise by an order of magnitude.
        y_small = nc.dram_tensor(
            "y", [1, 1], bf16 if fp8 else xdt, kind="ExternalOutput"
        ).ap()
    else:
        x_d = nc.dram_tensor("x", [128, R * 128], indt, kind="ExternalInput").ap()
        y_d = nc.dram_tensor("y", [128, R * 128], odt, kind="ExternalOutput").ap()

    from contextlib import ExitStack, nullcontext

    with tile.TileContext(nc) as tc, ExitStack() as ctx:
        hpool = ctx.enter_context(tc.tile_pool(name="hconst", bufs=1))
        xpool = ctx.enter_context(tc.tile_pool(name="xin", bufs=6))
        ypool = ctx.enter_context(tc.tile_pool(name="yout", bufs=6))
        mpool = ctx.enter_context(tc.tile_pool(name="mid", bufs=6))
        ps1pool = ctx.enter_context(
            tc.tile_pool(
                name="ps1", bufs=2,
                space=bass.MemorySpace.PSUM,
            )
        )
        ps2pool = ctx.enter_context(
            tc.tile_pool(name="ps2", bufs=2, space=bass.MemorySpace.PSUM)
        )
        if bench:
            dpool = ctx.enter_context(
                tc.tile_pool(name="dscratch", bufs=1, space=bass.MemorySpace.DRAM)
            )
            x_d = dpool.tile([128, R * 128], indt)
            y_d = dpool.tile([128, R * 128], odt)

        if fp8:
            h8t = hpool.tile([128, 128], f8)
            nc.sync.dma_start(h8t[:], h8_d[:])
            ht = hpool.tile([128, 128], mdt)
            nc.sync.dma_start(ht[:], h_d[:])
            rhs1 = h8t[:]       # H       (stage-1 moving operand, fp8e3)
            lhs2 = ht[:]        # H/128   (stage-2 stationary operand)
        else:
            ht = hpool.tile([128, 256], xdt)
            nc.sync.dma_start(ht[:], h_d[:])
            rhs1 = ht[:, 0:128]     # H       (stage-1 moving operand)
            lhs2 = ht[:, 128:256]   # H/128   (stage-2 stationary operand)

        copy_eng, quant_eng = _engine_split(fp16_out=f16out)
        if _os.environ.get("FWHT_POOL", "1") == "0":
            # legacy 2-engine split (DVE/ACT only)
            copy_eng = ["v" if g % 2 == 0 else "s" for g in range(NGRP_ALL)]
            quant_eng = [
                "v" if g % 16 in (1, 3, 5, 8, 10, 12, 15) else "s"
                for g in range(NGRP_ALL)
            ]

        def load_block(b):
            xt = xpool.tile([128, BLK * 128], indt, name="xt")
            if no_dma:
                nc.vector.tensor_copy(xt[:, 0:1], ht[:, 0:1])
            else:
                nc.sync.dma_start(
                    xt[:], x_d[:, b * BLK * 128 : (b + 1) * BLK * 128]
                )
            return xt

        def load_pair(p):
            # one 2-block (2 MB) in-DMA; callers slice per-block views
            xt = xpool.tile([128, 2 * BLK * 128], indt, name="xtp")
            if no_dma:
                nc.vector.tensor_copy(xt[:, 0:1], ht[:, 0:1])
            else:
                nc.sync.dma_start(
                    xt[:], x_d[:, p * 2 * BLK * 128 : (p + 1) * 2 * BLK * 128]
                )
            return [xt[:, 0 : BLK * 128], xt[:, BLK * 128 : 2 * BLK * 128]]

        def one_pass(preloaded, prefetch_next):
            """One full 256-row pass.  `preloaded` holds xt tiles for blocks
            0..PREF-1 (loaded during the previous pass's tail, or by the
            prologue).  Returns the next pass's preloaded tiles, emitted
            during this pass's tail so the SP DMA ring never idles across the
            pass boundary."""
            npre = 2 if MERGE_IN else PREF
            xts = list(preloaded) + [None] * (NBLK - npre)
            yts = [None] * NBLK
            sb1s = [None] * NGRP_ALL
            nxt = []
            ready_out = []

            def dma_out(b):
                nc.sync.dma_start(
                    y_d[:, b * BLK * 128 : (b + 1) * BLK * 128], yts[b][:]
                )

            def stage1(g):
                xt = xts[g // GPB]
                ps1 = ps1pool.tile([128, GRP * 128], f32)
                r0 = (g % GPB) * GRP
                for k in range(GRP):
                    nc.tensor.matmul(
                        ps1[:, k * 128 : (k + 1) * 128],
                        xt[:, (r0 + k) * 128 : (r0 + k + 1) * 128],
                        rhs1,
                        start=True,
                        stop=True,
                    )
                sb1 = mpool.tile([128, GRP * 128], mdt, name="sb1")
                # PSUM->SBUF casts split across DVE/ACT so engine time
                # balances (see _engine_split).  ACT uses mul(x,1.0) so every
                # ACT op is the same activation function (no table reloads).
                # fp8 scheme: mids are bf16; DVE drains via the strided
                # high-half bitcast view (0.993 ns/col, its fastest PSUM op;
                # bf16-RTZ truncation, ~+2e-3 rel err), ACT converts (RTN).
                ce = copy_eng[g]
                if ce == "v":
                    if fp8:
                        nc.vector.tensor_copy(
                            sb1[:], ps1[:].bitcast(bf16)[:, 1::2]
                        )
                    else:
                        nc.vector.tensor_copy(sb1[:], ps1[:])
                elif ce == "s":
                    nc.scalar.mul(sb1[:], ps1[:], 1.0)
                else:  # "vs" (fp8 only): DVE-hi first half, ACT-cv second.
                    # Single-level slices only: one getitem after bitcast,
                    # and direct tile slices for the f32/sb1 halves.
                    h = GRP * 128 // 2
                    nc.vector.tensor_copy(
                        sb1[:, 0:h], ps1[:].bitcast(bf16)[:, 1 : 2 * h : 2]
                    )
                    nc.scalar.mul(
                        sb1[:, h : GRP * 128], ps1[:, h : GRP * 128], 1.0
                    )
                sb1s[g] = sb1

            def stage2(g):
                b = g // GPB
                ps2 = ps2pool.tile([128, GRP * 128], f32)
                for hN in range(GRP * 128 // 512):
                    nc.tensor.matmul(
                        ps2[:, hN * 512 : (hN + 1) * 512],
                        lhs2,
                        sb1s[g][:, hN * 512 : (hN + 1) * 512],
                        start=True,
                        stop=True,
                    )
                sb1s[g] = None
                base = (g % GPB) * GRP * 128
                # output drains split across DVE/ACT (see _engine_split).
                # fp8 scheme: plain f32->fp16 copies (no int8 quant, ACT's
                # cheapest PSUM op); fp16 scheme: int8 quant with scale.
                e = quant_eng[g]
                if e == "v":
                    ys = yts[b][:, base : base + GRP * 128]
                    if f16out:
                        nc.vector.tensor_copy(ys, ps2[:])
                    else:
                        nc.vector.tensor_scalar_mul(ys, ps2[:], float(qscale))
                elif e == "s":
                    ys = yts[b][:, base : base + GRP * 128]
                    if f16out:
                        nc.scalar.mul(ys, ps2[:], 1.0)
                    else:
                        nc.scalar.mul(ys, ps2[:], float(qscale))
                else:  # "vs": half on each engine (single-level slices only)
                    h = GRP * 128 // 2
                    nc.vector.tensor_scalar_mul(
                        yts[b][:, base : base + h], ps2[:, 0:h], float(qscale)
                    )
                    nc.scalar.mul(
                        yts[b][:, base + h : base + GRP * 128],
                        ps2[:, h : GRP * 128],
                        float(qscale),
                    )

            if host2:
                # EXPERIMENTAL - do not use. Device = stage 1 only, host
                # applies the second H. CoreSim-exact but deterministically
                # corrupted on HW (mids come back ~3x too large plus noise;
                # suspected neuronxcc miscompile of int8-quant-from-8-MM-PSUM
                # under concurrent PE traffic). Default scheme is "fp16".
                for G in range(NGRP_ALL):
                    b = G // GPB
                    if G % GPB == 0:
                        if b + PREF < NBLK:
                            xts[b + PREF] = load_block(b + PREF)
                        elif prefetch_next and b + PREF - NBLK < PREF:
                            nxt.append(load_block(b + PREF - NBLK))
                        yts[b] = ypool.tile([128, BLK * 128], odt, name="yt")
                    xt = xts[b]
                    ps1 = ps1pool.tile([128, GRP * 128], f32)
                    r0 = (G % GPB) * GRP
                    for k in range(GRP):
                        nc.tensor.matmul(
                            ps1[:, k * 128 : (k + 1) * 128],
                            xt[:, (r0 + k) * 128 : (r0 + k + 1) * 128],
                            rhs1,
                            start=True,
                            stop=True,
                        )
                    ys = yts[b][
                        :, (G % GPB) * GRP * 128 : ((G % GPB) + 1) * GRP * 128
                    ]
                    if G % 2 == 0:
                        nc.vector.tensor_scalar_mul(ys, ps1[:], float(QSCALE2))
                    else:
                        nc.scalar.mul(ys, ps1[:], float(QSCALE2))
                    if G % GPB == GPB - 1 and not no_dma:
                        ready_out.append(b)
                        if len(ready_out) >= 2:
                            dma_out(ready_out.pop(0))
                while ready_out:
                    dma_out(ready_out.pop(0))
                return nxt
            # flat software pipeline over all groups: stage2 trails stage1 by
            # two groups so the PE never waits on the PSUM->SBUF cast.
            LAG = 2
            for G in range(NGRP_ALL + LAG):
                if G < NGRP_ALL:
                    b = G // GPB
                    if G % GPB == 0:
                        if MERGE_IN:
                            # pair-granular prefetch: at the first block of
                            # pair p, load pair p+1 (or next pass's pair 0)
                            if b % 2 == 0:
                                p = b // 2
                                if p + 1 < NBLK // 2:
                                    xts[2 * p + 2 : 2 * p + 4] = load_pair(p + 1)
                                elif prefetch_next:
                                    nxt.extend(load_pair(0))
                        elif b + PREF < NBLK:
                            xts[b + PREF] = load_block(b + PREF)
                        elif prefetch_next and b + PREF - NBLK < PREF:
                            nxt.append(load_block(b + PREF - NBLK))
                        yts[b] = ypool.tile([128, BLK * 128], odt, name="yt")
                    stage1(G)
                if G >= LAG:
                    Q = G - LAG
                    stage2(Q)
                    if Q % GPB == GPB - 1 and not no_dma:
                        # emit out(b) one block late: by the time SP reaches
                        # it, the quants it waits on are long done, so the SP
                        # ring never stalls and later in-DMAs issue on time.
                        ready_out.append(Q // GPB)
                        if len(ready_out) >= 2:
                            dma_out(ready_out.pop(0))
            while ready_out:
                dma_out(ready_out.pop(0))
            return nxt

        def body(npasses):
            if no_compute:
                yts = [None] * NBLK
                for b in range(min(PREF, NBLK)):
                    load_block(b)
                for b in range(NBLK):
                    if b + PREF < NBLK:
                        load_block(b + PREF)
                    yt = ypool.tile([128, BLK * 128], i8, name="yt")
                    nc.vector.tensor_copy(yt[:, 0:1], ht[:, 0:1])
                    if not no_dma:
                        nc.sync.dma_start(
                            y_d[:, b * BLK * 128 : (b + 1) * BLK * 128], yt[:]
                        )
                return
            if MERGE_IN:
                pre = load_pair(0)
            else:
                pre = [load_block(b) for b in range(min(PREF, NBLK))]
            for p in range(npasses):
                pre = one_pass(pre, p < npasses - 1)

        loop_cm = (
            tc.For_i(
                0, repeat, 1,
                # only the PE body exceeds one IRAM block at unroll=4; hints
                # on engines whose body fits are a net loss (~0.16us/edge)
                hint_engines=(mybir.EngineType.PE,),
                staggered_reset=_os.environ.get("FWHT_STAG", "0") == "1",
            )
            if bench
            else nullcontext()
        )
        with loop_cm:
            body(unroll if bench else 1)

        if bench:
            nc.sync.dma_start(y_small[:], ht[0:1, 0:1])

    nc.compile()
    return nc


SCHEME = _os.environ.get("FWHT_SCHEME", "fp8")


def kernel(**inputs) -> np.ndarray:
    global LAST_RESULTS
    # NTFF tracing is unavailable under this axon tunnel (antenv.axon_hooks
    # missing) and would crash run_bass_kernel_spmd if BASS_TRACE leaked in.
    _os.environ["BASS_NEVER_TRACE"] = "1"
    x = np.asarray(inputs["x"])
    B, C, N = x.shape
    assert (B, C, N) == (32, 64, 16384)

    if "nc" not in _cache:
        _cache["nc"] = _build(scheme=SCHEME)
    nc = _cache["nc"]

    import ml_dtypes
    if SCHEME == "fp8":
        np_xdt = ml_dtypes.float8_e3m4
    elif SCHEME == "fp16":
        np_xdt = np.float16
    else:
        np_xdt = ml_dtypes.bfloat16

    hmap = _h_inputs(SCHEME)
    # [2048 rows, i, j] -> per-core [i, (r j)] so every DMA line is contiguous
    xh = x.reshape(B * C, 128, 128).astype(np_xdt)
    in_maps = [
        {
            "x": np.ascontiguousarray(
                xh[c * R : (c + 1) * R].transpose(1, 0, 2)
            ).reshape(128, R * 128),
            **hmap,
        }
        for c in range(N_CORES)
    ]
    res = run_bass_kernel_spmd(nc, in_maps, core_ids=list(range(N_CORES)))
    LAST_RESULTS = res

    out = np.empty((B * C, 16384), dtype=np.float32)
    if SCHEME == "host2":
        # y holds int8 mids M[j, (r a)] = (H @ X_r)[a, j]; finish on host:
        # Y_r[a, v] = sum_j M[j, r, a] H[j, v] / 128
        Hm = (_hadamard128() / np.float32(128.0)) * np.float32(MID_AMP / 127.0)
        for c in range(N_CORES):
            m = res.results[c]["y"].reshape(128, R * 128).astype(np.float32)
            g = Hm.T @ m  # [v, (r a)]
            out[c * R : (c + 1) * R] = (
                g.reshape(128, R, 128).transpose(1, 2, 0).reshape(R, 16384)
            )
    else:
        # fp8 scheme outputs fp16 y directly (no quant scale); fp16 scheme
        # outputs int8 with the QAMP scale.
        f16out = SCHEME == "fp8" and _os.environ.get("FWHT_FP16OUT", "1") == "1"
        deq = np.float32(
            1.0 if f16out else (QAMP8 if SCHEME == "fp8" else QAMP) / 127.0
        )
        for c in range(N_CORES):
            yc = res.results[c]["y"].reshape(128, R, 128)  # [b, r, a] = Y_r[a, b]
            yr = yc.transpose(1, 2, 0).astype(np.float32) * deq  # [r, a, b]
            out[c * R : (c + 1) * R] = yr.reshape(R, 16384)
    return out.reshape(B, C, N)

